# revision 12
# baseline (speedup 1.0000x reference)
"""Trainium2 Bass kernel for nn_CrossChannelAttention.

Reference computation (per batch b, pixel p, with C=128 channels, NUMS=16
groups of HEADS=8 channels, OUT=256):
    fm[g,p]  = relu(sum_h W1[g,h] * x[8g+h, p] + b1[g])          # [16, P]
    feat[(g,d), p] = fm[g,p] * x[d,p]                            # [2048, P]
    out[o,p] = sum_c W2[o,c] * feat[c,p] + b2[o]                 # [256, P]

Strategy: data-parallel over batch B=8 across the 8 NeuronCores (one batch
image per core, params replicated).  Per core:
  - prologue: zero-matmul warmup into the fm PSUM banks (keeps the PE busy
    from ~5.5us and flips the HAM clock gate to 2.4 GHz before real work),
    x loaded in chunks (first chunk split for latency), fm via small
    matmuls + relu on the scalar engine.
  - fm row g is broadcast to 128 partitions in [128,1024] chunks, split
    between DRAM->SBUF broadcast DMAs (wide shapes fan out across all 16 DMA
    engines; triggers round-robin over sync/scalar/tensor queues) and
    gpsimd.partition_broadcast reading fm_sb rows directly (rows 0-15 live
    in Q7 core 0's partition slice, so no DRAM round trip).
  - feat = x * fm_rep on the vector engine as a pure-SBUF bf16 multiply.
  - the PE runs the 256 accumulating K=128 main matmuls bf16 N=512; for the
    last pixel group the final 4 g-steps are bank-major so output banks
    drain (scalar activation + DMA) under the remaining matmuls.
  - out is staged and stored as bf16 (halves output DMA bytes; adds ~0.1%
    error against a 2e-2 budget), upcast to f32 on host.
Accuracy: bf16 matmuls with fp32 PSUM accumulation; rel err ~4e-3.
"""

import numpy as np
import ml_dtypes

import concourse.bacc as bacc
import concourse.tile as tile
from concourse import mybir
from concourse.bass_utils import run_bass_kernel_spmd

F32 = mybir.dt.float32
BF16 = mybir.dt.bfloat16

B, C, H, W = 8, 128, 64, 64
NUMS, HEADS, OUT = 16, 8, 256
P = H * W          # 4096 pixels per image
PB = 512           # pixel block (one PSUM bank of fp32)
NPB = P // PB      # 8 pixel blocks
GRP = 1024         # broadcast chunk (2 pixel blocks)
NGRP = P // GRP    # 4 broadcast groups
N_CORES = 8
LOOKAHEAD = 10     # broadcast/feat pipeline depth (in (g,k) units) ahead of mains
ZWARM = 3          # zero warmup matmuls per fm PSUM bank
GPSIMD_GS = {3, 7, 11, 14}         # groups replicated via gpsimd.partition_broadcast
TAIL_BM = 4                        # last-k: bank-major span (final g steps)

_CACHE = {}


def _build():
    nc = bacc.Bacc("TRN2", target_bir_lowering=False, debug=False,
                   num_devices=N_CORES)

    x_d = nc.dram_tensor("x", [C, P], BF16, kind="ExternalInput")
    w1s_d = nc.dram_tensor("w1s", [C, NUMS], BF16, kind="ExternalInput")
    w2t_d = nc.dram_tensor("w2t", [C, NUMS * OUT], BF16, kind="ExternalInput")
    b1_d = nc.dram_tensor("b1c", [NUMS, 1], F32, kind="ExternalInput")
    b2_d = nc.dram_tensor("b2c", [C, 2], F32, kind="ExternalInput")
    out_d = nc.dram_tensor("out", [OUT, P], BF16, kind="ExternalOutput")

    relu = mybir.ActivationFunctionType.Relu
    ident = mybir.ActivationFunctionType.Identity
    mult = mybir.AluOpType.mult

    with tile.TileContext(nc) as tc:
        with (
            tc.tile_pool(name="const", bufs=1) as cpool,
            tc.tile_pool(name="fmrow", bufs=1) as frp,
            tc.tile_pool(name="xbp", bufs=1) as xbp,
            tc.tile_pool(name="repp", bufs=22) as repp,
            tc.tile_pool(name="feat", bufs=2 * LOOKAHEAD + 2) as featp,
            tc.tile_pool(name="osb", bufs=4) as osb,
            tc.tile_pool(name="ps", bufs=8, space="PSUM") as ps,
            tc.tile_pool(name="dr", bufs=4, space="DRAM") as drp,
        ):
            # ---- prologue: params + x + zero-warmup + fm ----
            w1s_t = cpool.tile([C, NUMS], BF16)
            nc.sync.dma_start(w1s_t[:], w1s_d[:])
            b1_t = cpool.tile([NUMS, 1], F32)
            nc.gpsimd.dma_start(b1_t[:], b1_d[:])
            b2_t = cpool.tile([C, 2], F32)
            nc.gpsimd.dma_start(b2_t[:], b2_d[:])
            zeros_t = cpool.tile([C, PB], BF16)
            nc.gpsimd.memset(zeros_t[:], 0)

            # x chunks; first GRP chunk split in two for latency
            x2s = []
            xq = [nc.sync, nc.scalar, nc.sync, nc.scalar]
            for k in range(NGRP):
                x2 = xbp.tile([C, GRP], BF16, tag=f"x2_{k}", name=f"x2_{k}")
                x2s.append(x2)
                gx = slice(k * GRP, (k + 1) * GRP)
                if k == 0:
                    nc.sync.dma_start(x2[:, 0:PB], x_d[:, 0:PB])
                    nc.scalar.dma_start(x2[:, PB:GRP], x_d[:, PB:GRP])
                else:
                    xq[k].dma_start(x2[:], x_d[:, gx])

            # w2t halves on sync + scalar queues (1MB total)
            w2t_t = cpool.tile([C, NUMS * OUT], BF16)
            HALF = NUMS * OUT // 2
            nc.sync.dma_start(w2t_t[:, 0:HALF], w2t_d[:, 0:HALF])
            nc.scalar.dma_start(w2t_t[:, HALF:], w2t_d[:, HALF:])

            # fm: per pixel block, zero-warmup MMs then the real one
            fm_sb = cpool.tile([NUMS, P], BF16)
            fm_drs = [drp.tile([NUMS, GRP], BF16, tag=f"fmdr{k}",
                               name=f"fmdr{k}")
                      for k in range(NGRP)]
            fmrows = {}
            for pb in range(NPB):
                k, half = pb // 2, pb % 2
                px = slice(pb * PB, (pb + 1) * PB)
                hx = slice(half * PB, (half + 1) * PB)
                ps_fm = ps.tile([NUMS, PB], F32, tag="ps", name=f"psfm{pb}")
                for z in range(ZWARM):
                    nc.tensor.matmul(ps_fm[:], w1s_t[:], zeros_t[:],
                                     start=(z == 0), stop=False)
                nc.tensor.matmul(ps_fm[:], w1s_t[:], x2s[k][:, hx],
                                 start=False, stop=True)
                nc.scalar.activation(fm_sb[:, px], ps_fm[:], relu,
                                     bias=b1_t[:])
                if half == 1:
                    gx = slice(k * GRP, (k + 1) * GRP)
                    nc.scalar.dma_start(fm_drs[k][:], fm_sb[:, gx])
                    for j, g in enumerate(sorted(GPSIMD_GS)):
                        fr = frp.tile([1, GRP], BF16, tag=f"fr{g}_{k}",
                                      name=f"fr{g}_{k}")
                        eng = nc.sync if (j + k) % 2 else nc.scalar
                        eng.dma_start(fr[:], fm_drs[k][g:g + 1, :])
                        fmrows[(g, k)] = fr

            # ---- replication + feat, pipelined ahead of the mains ----
            nbc = [0]
            bq = [nc.sync, nc.scalar]

            def emit_ft(g, k):
                rep = repp.tile([C, GRP], BF16, tag="rep", name=f"rep{g}_{k}")
                gx = slice(k * GRP, (k + 1) * GRP)
                if g in GPSIMD_GS:
                    fr = fmrows.pop((g, k))
                    nc.gpsimd.partition_broadcast(rep[:], fr[0:1, :])
                else:
                    eng = bq[nbc[0] % 2]
                    nbc[0] += 1
                    eng.dma_start(rep[:],
                                  fm_drs[k][g:g + 1, :].broadcast_to((C, GRP)))
                ft = featp.tile([C, GRP], BF16, tag="ft", name=f"ft{g}_{k}")
                nc.vector.tensor_tensor(ft[:], x2s[k][:], rep[:], op=mult)
                fts[(g, k)] = ft

            fts = {}
            todo = [(g, k) for k in range(NGRP) for g in range(NUMS)]
            for i in range(LOOKAHEAD):
                emit_ft(*todo[i])

            def w2slc(g, oc):
                return w2t_t[:, (2 * g + oc) * C:(2 * g + oc + 1) * C]

            def drain(pbb, oc, k):
                px = slice(pbb * PB, (pbb + 1) * PB)
                o = osb.tile([C, PB], BF16, tag="osb", name=f"o{pbb}_{oc}")
                nc.scalar.activation(o[:], pso.pop((pbb, oc))[:],
                                     ident, bias=b2_t[:, oc:oc + 1])
                nc.sync.dma_start(out_d[oc * C:(oc + 1) * C, px], o[:])

            pso = {}
            for i, (g, k) in enumerate(todo):
                if i + LOOKAHEAD < len(todo):
                    emit_ft(*todo[i + LOOKAHEAD])
                last_k = (k == NGRP - 1)
                if last_k and g == NUMS - TAIL_BM:
                    # bank-major tail: finish each bank's remaining g's in
                    # turn so its ACT+DMA overlaps the other banks' matmuls
                    for pbb in (2 * k, 2 * k + 1):
                        half = pbb - 2 * k
                        hx = slice(half * PB, (half + 1) * PB)
                        for oc in range(2):
                            for gg in range(NUMS - TAIL_BM, NUMS):
                                ft = fts[(gg, k)]
                                nc.tensor.matmul(
                                    pso[(pbb, oc)][:], w2slc(gg, oc),
                                    ft[:, hx], start=False,
                                    stop=(gg == NUMS - 1))
                            drain(pbb, oc, k)
                    for gg in range(NUMS - TAIL_BM, NUMS):
                        fts.pop((gg, k))
                    break
                ft = fts.pop((g, k))
                if g == 0:
                    for pbb in (2 * k, 2 * k + 1):
                        for oc in range(2):
                            t = ps.tile([C, PB], F32, tag="ps",
                                        name=f"pso{pbb}_{oc}")
                            pso[(pbb, oc)] = t
                for half in range(2):
                    pb = 2 * k + half
                    hx = slice(half * PB, (half + 1) * PB)
                    nc.tensor.matmul(pso[(pb, 0)][:], w2slc(g, 0),
                                     ft[:, hx], start=(g == 0),
                                     stop=False)
                    nc.tensor.matmul(pso[(pb, 1)][:], w2slc(g, 1),
                                     ft[:, hx], start=(g == 0),
                                     stop=False)
                if g == NUMS - 1:
                    for pbb in (2 * k, 2 * k + 1):
                        for oc in range(2):
                            drain(pbb, oc, k)

    nc.compile()
    return nc


def _prep_params(W1, b1, W2, b2):
    bf = ml_dtypes.bfloat16
    # w1s[c, g] = W1[g, c - 8g] for 8g <= c < 8(g+1), else 0
    w1s = np.zeros((C, NUMS), dtype=bf)
    for g in range(NUMS):
        w1s[g * HEADS:(g + 1) * HEADS, g] = W1[g].astype(bf)
    # w2t[k, (g*2+oc)*128 + m] = W2[oc*128 + m, g*128 + k]
    w2t = (
        np.asarray(W2, dtype=np.float32)
        .reshape(2, C, NUMS, C)          # [oc, m, g, k]
        .transpose(3, 2, 0, 1)           # [k, g, oc, m]
        .reshape(C, NUMS * OUT)
        .astype(bf)
    )
    b1c = np.asarray(b1, dtype=np.float32).reshape(NUMS, 1).copy()
    b2c = np.asarray(b2, dtype=np.float32).reshape(2, C).T.copy()
    return w1s, w2t, b1c, b2c


def kernel(x, W1, b1, W2, b2, _trace=False, _trace_kwargs=None):
    if "nc" not in _CACHE:
        _CACHE["nc"] = _build()
    nc = _CACHE["nc"]

    w1s, w2t, b1c, b2c = _prep_params(W1, b1, W2, b2)
    xs = np.ascontiguousarray(
        np.asarray(x, dtype=np.float32).reshape(B, C, P).astype(ml_dtypes.bfloat16))
    in_maps = [
        {"x": xs[b_], "w1s": w1s, "w2t": w2t, "b1c": b1c, "b2c": b2c}
        for b_ in range(N_CORES)
    ]
    kwargs = {}
    if _trace:
        kwargs["trace"] = True
        kwargs.update(_trace_kwargs or {})
    res = run_bass_kernel_spmd(nc, in_maps, core_ids=list(range(N_CORES)),
                               **kwargs)
    out = np.stack([np.asarray(res.results[b_]["out"], dtype=np.float32)
                    for b_ in range(N_CORES)])
    out = out.reshape(B, OUT, H, W)
    if _trace:
        _CACHE["last_result"] = res
    return out


# revision 13
# speedup vs baseline: 1.0246x; 1.0246x over previous
"""Trainium2 Bass kernel for nn_CrossChannelAttention.

Reference computation (per batch b, pixel p, with C=128 channels, NUMS=16
groups of HEADS=8 channels, OUT=256):
    fm[g,p]  = relu(sum_h W1[g,h] * x[8g+h, p] + b1[g])          # [16, P]
    feat[(g,d), p] = fm[g,p] * x[d,p]                            # [2048, P]
    out[o,p] = sum_c W2[o,c] * feat[c,p] + b2[o]                 # [256, P]

Data-parallel over batch B=8 across the 8 NeuronCores.  Per core the PE
floor is 256 accumulating K=128 bf16 N=512 matmuls (~56us warm), so the
kernel is organised so every other engine hides under that stream:
  - emission order == queue order: the critical chain (x first chunk -> fm
    matmul -> relu -> fm DRAM write -> broadcast -> DVE multiply) is emitted
    before any bulk transfer; w2t/x bulk loads are chopped and interleaved
    behind it.
  - zero-matmuls accumulate into the fm PSUM banks before the real x
    arrives: PE busy from ~8us -> HAM clock-gate warm before the mains.
  - fm rows are replicated to 128 partitions by DRAM-broadcast DMAs
    (sync/scalar queues) and gpsimd.partition_broadcast (3 groups/chunk),
    feat = x * fm_rep on the vector engine (bf16 2x), PE never waits.
  - two 4-bank PSUM pools alternate between pixel groups so a group's
    accumulation never waits on the previous group's drain.
  - fm for the last 4 pixel blocks is computed mid-stream (no head-of-line
    block on late x chunks); output is staged bf16 (halves the store DMA)
    and the last pixel group drains bank-major under the final matmuls.
Accuracy: bf16 matmuls with fp32 PSUM accumulation; rel err ~4e-3.
"""

import numpy as np
import ml_dtypes

import concourse.bacc as bacc
import concourse.tile as tile
from concourse import mybir
from concourse.bass_utils import run_bass_kernel_spmd

F32 = mybir.dt.float32
BF16 = mybir.dt.bfloat16

B, C, H, W = 8, 128, 64, 64
NUMS, HEADS, OUT = 16, 8, 256
P = H * W          # 4096 pixels per image
PB = 512           # pixel block (one PSUM bank of fp32)
NPB = P // PB      # 8 pixel blocks
GRP = 1024         # broadcast chunk (2 pixel blocks)
NGRP = P // GRP    # 4 broadcast groups
N_CORES = 8
LOOKAHEAD = 10     # feat pipeline depth (units) ahead of the mains
LA_G = 14          # extra lead for gpsimd-broadcast reps
ZW = [3, 3, 6, 6, 0, 0, 0, 0]      # zero warmup matmuls per fm PSUM bank
GPSIMD_GS = {3, 8, 13}             # groups replicated via gpsimd per chunk
TAIL_BM = 4                        # last-k: bank-major span (final g steps)
W2CH = 4                           # w2t load chunks

_CACHE = {}


def _build():
    nc = bacc.Bacc("TRN2", target_bir_lowering=False, debug=False,
                   num_devices=N_CORES)

    x_d = nc.dram_tensor("x", [C, P], BF16, kind="ExternalInput")
    w1s_d = nc.dram_tensor("w1s", [C, NUMS], BF16, kind="ExternalInput")
    w2t_d = nc.dram_tensor("w2t", [C, NUMS * OUT], BF16, kind="ExternalInput")
    b1_d = nc.dram_tensor("b1c", [NUMS, 1], F32, kind="ExternalInput")
    b2_d = nc.dram_tensor("b2c", [C, 2], F32, kind="ExternalInput")
    out_d = nc.dram_tensor("out", [OUT, P], BF16, kind="ExternalOutput")

    relu = mybir.ActivationFunctionType.Relu
    ident = mybir.ActivationFunctionType.Identity
    mult = mybir.AluOpType.mult

    with tile.TileContext(nc) as tc:
        with (
            tc.tile_pool(name="const", bufs=1) as cpool,
            tc.tile_pool(name="fmrow", bufs=1) as frp,
            tc.tile_pool(name="xbp", bufs=1) as xbp,
            tc.tile_pool(name="repp", bufs=22) as repp,
            tc.tile_pool(name="feat", bufs=2 * LOOKAHEAD + 2) as featp,
            tc.tile_pool(name="osb", bufs=4) as osb,
            tc.tile_pool(name="psA", bufs=4, space="PSUM") as psA,
            tc.tile_pool(name="psB", bufs=4, space="PSUM") as psB,
            tc.tile_pool(name="dr", bufs=4, space="DRAM") as drp,
        ):
            # ---- phase 0: minimal loads for the critical chain ----
            w1s_t = cpool.tile([C, NUMS], BF16)
            nc.sync.dma_start(w1s_t[:], w1s_d[:])
            zeros_t = cpool.tile([C, PB], BF16)
            nc.vector.memset(zeros_t[:], 0)
            b1_t = cpool.tile([NUMS, 1], F32)
            nc.gpsimd.dma_start(b1_t[:], b1_d[:])
            b2_t = cpool.tile([C, 2], F32)
            nc.gpsimd.dma_start(b2_t[:], b2_d[:])

            x2s = [xbp.tile([C, GRP], BF16, tag=f"x2_{k}", name=f"x2_{k}")
                   for k in range(NGRP)]
            nc.sync.dma_start(x2s[0][:, 0:PB], x_d[:, 0:PB])
            nc.scalar.dma_start(x2s[0][:, PB:GRP], x_d[:, PB:GRP])

            w2t_t = cpool.tile([C, NUMS * OUT], BF16)
            fm_sb = cpool.tile([NUMS, P], BF16)
            fm_drs = [drp.tile([NUMS, GRP], BF16, tag=f"fmdr{k}",
                               name=f"fmdr{k}")
                      for k in range(NGRP)]
            fmrows = {}

            def emit_fm_pb(pb, pool):
                k, half = pb // 2, pb % 2
                px = slice(pb * PB, (pb + 1) * PB)
                hx = slice(half * PB, (half + 1) * PB)
                ps_fm = pool.tile([NUMS, PB], F32, tag="ps", name=f"psfm{pb}")
                for z in range(ZW[pb]):
                    nc.tensor.matmul(ps_fm[:], w1s_t[:], zeros_t[:],
                                     start=(z == 0), stop=False)
                nc.tensor.matmul(ps_fm[:], w1s_t[:], x2s[k][:, hx],
                                 start=(ZW[pb] == 0), stop=True)
                nc.scalar.activation(fm_sb[:, px], ps_fm[:], relu,
                                     bias=b1_t[:])

            def emit_fm_k(k, pool):
                emit_fm_pb(2 * k, pool)
                emit_fm_pb(2 * k + 1, pool)
                gx = slice(k * GRP, (k + 1) * GRP)
                nc.scalar.dma_start(fm_drs[k][:], fm_sb[:, gx])
                for j, g in enumerate(sorted(GPSIMD_GS)):
                    fr = frp.tile([1, GRP], BF16, tag=f"fr{g}_{k}",
                                  name=f"fr{g}_{k}")
                    eng = nc.sync if (j + k) % 2 else nc.scalar
                    eng.dma_start(fr[:], fm_drs[k][g:g + 1, :])
                    fmrows[(g, k)] = fr

            # ---- replication + feat emission helpers ----
            nbc = [0]
            bq = [nc.sync, nc.scalar]
            reps = {}
            fts = {}

            def emit_rep(g, k):
                rep = repp.tile([C, GRP], BF16, tag="rep", name=f"rep{g}_{k}")
                if g in GPSIMD_GS:
                    nc.gpsimd.partition_broadcast(rep[:],
                                                  fmrows.pop((g, k))[0:1, :])
                else:
                    eng = bq[nbc[0] % 2]
                    nbc[0] += 1
                    eng.dma_start(rep[:],
                                  fm_drs[k][g:g + 1, :].broadcast_to((C, GRP)))
                reps[(g, k)] = rep

            def emit_ft(g, k):
                if (g, k) not in reps:
                    emit_rep(g, k)
                ft = featp.tile([C, GRP], BF16, tag="ft", name=f"ft{g}_{k}")
                nc.vector.tensor_tensor(ft[:], x2s[k][:], reps.pop((g, k))[:],
                                        op=mult)
                fts[(g, k)] = ft

            HCH = NUMS * OUT // W2CH

            def load_w2t(c, eng):
                cx = slice(c * HCH, (c + 1) * HCH)
                eng.dma_start(w2t_t[:, cx], w2t_d[:, cx])

            # ---- pre-main script: critical chain first, bulk behind ----
            emit_fm_k(0, psA)                       # fm pb0, pb1 + fm_dr0
            nc.scalar.dma_start(x2s[1][:], x_d[:, GRP:2 * GRP])
            emit_ft(0, 0)
            emit_ft(1, 0)
            load_w2t(0, nc.sync)
            emit_ft(2, 0)
            emit_fm_k(1, psA)                       # fm pb2, pb3 + fm_dr1
            nc.sync.dma_start(x2s[2][:], x_d[:, 2 * GRP:3 * GRP])
            emit_ft(3, 0)
            emit_ft(4, 0)
            load_w2t(1, nc.scalar)
            emit_ft(5, 0)
            emit_ft(6, 0)
            nc.scalar.dma_start(x2s[3][:], x_d[:, 3 * GRP:4 * GRP])
            for g in range(7, LOOKAHEAD):
                emit_ft(g, 0)

            def w2slc(g, oc):
                return w2t_t[:, (2 * g + oc) * C:(2 * g + oc + 1) * C]

            def drain(pbb, oc):
                px = slice(pbb * PB, (pbb + 1) * PB)
                o = osb.tile([C, PB], BF16, tag="osb", name=f"o{pbb}_{oc}")
                nc.scalar.activation(o[:], pso.pop((pbb, oc))[:],
                                     ident, bias=b2_t[:, oc:oc + 1])
                nc.gpsimd.dma_start(out_d[oc * C:(oc + 1) * C, px], o[:])

            inserts = {
                2: lambda: emit_fm_k(2, psB),
                4: lambda: load_w2t(2, nc.sync),
                6: lambda: emit_fm_k(3, psB),
                8: lambda: load_w2t(3, nc.scalar),
            }

            # ---- main stream ----
            pso = {}
            todo = [(g, k) for k in range(NGRP) for g in range(NUMS)]
            for i, (g, k) in enumerate(todo):
                if i in inserts:
                    inserts[i]()
                if i + LA_G < len(todo):
                    g2, k2 = todo[i + LA_G]
                    if g2 in GPSIMD_GS:
                        emit_rep(g2, k2)
                if i + LOOKAHEAD < len(todo):
                    emit_ft(*todo[i + LOOKAHEAD])
                last_k = (k == NGRP - 1)
                if last_k and g == NUMS - TAIL_BM:
                    # bank-major tail: drain each bank under the others' MMs
                    for pbb in (2 * k, 2 * k + 1):
                        half = pbb - 2 * k
                        hx = slice(half * PB, (half + 1) * PB)
                        for oc in range(2):
                            for gg in range(NUMS - TAIL_BM, NUMS):
                                nc.tensor.matmul(
                                    pso[(pbb, oc)][:], w2slc(gg, oc),
                                    fts[(gg, k)][:, hx], start=False,
                                    stop=(gg == NUMS - 1))
                            drain(pbb, oc)
                    break
                ft = fts.pop((g, k)) if not last_k else fts[(g, k)]
                if not last_k and g < NUMS - TAIL_BM:
                    pass
                if g == 0:
                    pool = psA if k % 2 == 0 else psB
                    for pbb in (2 * k, 2 * k + 1):
                        for oc in range(2):
                            pso[(pbb, oc)] = pool.tile(
                                [C, PB], F32, tag="ps", name=f"pso{pbb}_{oc}")
                for half in range(2):
                    pb = 2 * k + half
                    hx = slice(half * PB, (half + 1) * PB)
                    nc.tensor.matmul(pso[(pb, 0)][:], w2slc(g, 0),
                                     ft[:, hx], start=(g == 0), stop=False)
                    nc.tensor.matmul(pso[(pb, 1)][:], w2slc(g, 1),
                                     ft[:, hx], start=(g == 0), stop=False)
                if g == NUMS - 1:
                    for pbb in (2 * k, 2 * k + 1):
                        for oc in range(2):
                            drain(pbb, oc)

    nc.compile()
    return nc


def _prep_params(W1, b1, W2, b2):
    bf = ml_dtypes.bfloat16
    # w1s[c, g] = W1[g, c - 8g] for 8g <= c < 8(g+1), else 0
    w1s = np.zeros((C, NUMS), dtype=bf)
    for g in range(NUMS):
        w1s[g * HEADS:(g + 1) * HEADS, g] = W1[g].astype(bf)
    # w2t[k, (g*2+oc)*128 + m] = W2[oc*128 + m, g*128 + k]
    w2t = (
        np.asarray(W2, dtype=np.float32)
        .reshape(2, C, NUMS, C)          # [oc, m, g, k]
        .transpose(3, 2, 0, 1)           # [k, g, oc, m]
        .reshape(C, NUMS * OUT)
        .astype(bf)
    )
    b1c = np.asarray(b1, dtype=np.float32).reshape(NUMS, 1).copy()
    b2c = np.asarray(b2, dtype=np.float32).reshape(2, C).T.copy()
    return w1s, w2t, b1c, b2c


def kernel(x, W1, b1, W2, b2, _trace=False, _trace_kwargs=None):
    if "nc" not in _CACHE:
        _CACHE["nc"] = _build()
    nc = _CACHE["nc"]

    w1s, w2t, b1c, b2c = _prep_params(W1, b1, W2, b2)
    xs = np.ascontiguousarray(
        np.asarray(x, dtype=np.float32).reshape(B, C, P).astype(ml_dtypes.bfloat16))
    in_maps = [
        {"x": xs[b_], "w1s": w1s, "w2t": w2t, "b1c": b1c, "b2c": b2c}
        for b_ in range(N_CORES)
    ]
    kwargs = {}
    if _trace:
        kwargs["trace"] = True
        kwargs.update(_trace_kwargs or {})
    res = run_bass_kernel_spmd(nc, in_maps, core_ids=list(range(N_CORES)),
                               **kwargs)
    out = np.stack([np.asarray(res.results[b_]["out"], dtype=np.float32)
                    for b_ in range(N_CORES)])
    out = out.reshape(B, OUT, H, W)
    if _trace:
        _CACHE["last_result"] = res
    return out
